# revision 8
# baseline (speedup 1.0000x reference)
"""Sliding-window causal attention (B=4,H=16,N=2048,D=64, window=256) on 8 trn2 cores.

Sharding: B*H = 64 independent (b,h) attention slices, 8 per core.

Per-core algorithm (S^T "key-major" layout):
  - load Q,K,V fp32, cast to bf16; transpose Q,K to [D, N] via DMA xbar.
  - per key-tile kt (128 keys): S^T[k, j] = sum_d K[k,d] Q[q,d] for the 384
    queries q = kt*128 + j that can attend these keys (matmul, fp32 PSUM).
  - P^T = exp(S^T / 8) via ScalarE (no max-subtraction needed: |scores/8| <~ 6).
  - band masks (causal + window) applied as two strided bf16 multiplies.
  - O^T[d, q] = sum_k Vaug[k, d] P^T[k, q] accumulated over overlapping
    query-windows in PSUM; Vaug has a ones column so row 64 = softmax denom.
  - transpose O^T back via DMA xbar, normalize with reciprocal * broadcast mul.
"""

import numpy as np

B, H, N, D = 4, 16, 2048, 64
WINDOW = 256
NCORES = 8
PER = (B * H) // NCORES  # 8 bh slices per core
NT = N // 128  # 16 key/query tiles per slice
SCALE = 1.0 / 8.0  # 1/sqrt(64)

_CACHE = {}


def _build():
    import concourse.bass as bass
    import concourse.mybir as mybir
    import concourse.tile as tile
    import concourse.bacc as bacc

    f32 = mybir.dt.float32
    bf16 = mybir.dt.bfloat16
    Exp = mybir.ActivationFunctionType.Exp

    nc = bacc.Bacc("TRN2", debug=False)
    q_d = nc.dram_tensor("q", [PER, N, D], f32, kind="ExternalInput")
    k_d = nc.dram_tensor("k", [PER, N, D], f32, kind="ExternalInput")
    v_d = nc.dram_tensor("v", [PER, N, D], f32, kind="ExternalInput")
    o_d = nc.dram_tensor("o", [PER, N, D], f32, kind="ExternalOutput")

    # DRAM views. For Q/K use the xbar-pairing layout: tile t = e*8 + j is
    # staged at [p, j, e, d] so each [128, (e d)=128] block feeds one DMA
    # xbar transpose, whose output partitions 0:64 hold tile j's dims and
    # 64:128 hold tile (j+8)'s dims. One DMA per half (e) keeps APs 3-dim.
    def half_view(t):  # (ap for e=0, ap for e=1) each [B, p, j, d]
        a = t.ap().rearrange("B (e j p) d -> B e p j d", e=2, p=128)
        return a[:, 0:1], a[:, 1:2]

    qv0, qv1 = half_view(q_d)
    kv0, kv1 = half_view(k_d)
    vv = v_d.ap().rearrange("B (t p) d -> B p t d", p=128)
    ov = o_d.ap().rearrange("B (t p) d -> B p t d", p=128)

    with tile.TileContext(nc) as tc:
        with (
            tc.tile_pool(name="const", bufs=1) as cpool,
            tc.tile_pool(name="stage", bufs=2) as iopool,
            tc.tile_pool(name="work", bufs=2) as wpool,
            tc.tile_pool(name="pbuf", bufs=2) as ppool,
            tc.tile_pool(name="obuf", bufs=2) as opool,
            tc.tile_pool(name="spsum", bufs=3, space="PSUM") as spool,
            tc.tile_pool(name="opsum", bufs=1, space="PSUM") as opsum_pool,
        ):
            # Band masks. S^T tile (kt): partition ki (key kt*128+ki), col j
            # (query kt*128+j), j in [0, 384). Valid iff ki <= j <= ki + 256.
            # cols [0,128): keep j >= ki; cols [256,384): keep j-256 <= ki.
            m1 = cpool.tile([128, 128], bf16, name="m1")
            m2 = cpool.tile([128, 128], bf16, name="m2")
            nc.gpsimd.memset(m1[:], 1.0)
            nc.gpsimd.affine_select(
                out=m1[:], in_=m1[:], compare_op=mybir.AluOpType.is_ge,
                fill=0.0, base=0, pattern=[[1, 128]], channel_multiplier=-1,
            )  # keep where (-ki + j) >= 0
            nc.gpsimd.memset(m2[:], 1.0)
            nc.gpsimd.affine_select(
                out=m2[:], in_=m2[:], compare_op=mybir.AluOpType.is_ge,
                fill=0.0, base=0, pattern=[[-1, 128]], channel_multiplier=1,
            )  # keep where (ki - j) >= 0

            for b in range(PER):
                # ---- load + cast + transpose ----
                qf = iopool.tile([128, 8, 2, D], f32, tag="qf")
                kf = iopool.tile([128, 8, 2, D], f32, tag="kf")
                vf = iopool.tile([128, NT, D], f32, tag="vf")
                nc.sync.dma_start(qf[:, :, 0:1, :], qv0[b : b + 1])
                nc.sync.dma_start(qf[:, :, 1:2, :], qv1[b : b + 1])
                nc.sync.dma_start(kf[:, :, 0:1, :], kv0[b : b + 1])
                nc.sync.dma_start(kf[:, :, 1:2, :], kv1[b : b + 1])
                nc.sync.dma_start(vf[:], vv[b : b + 1])

                qb = wpool.tile([128, 8, 2, D], bf16, tag="qb")
                kb = wpool.tile([128, 8, 2, D], bf16, tag="kb")
                nc.vector.tensor_copy(qb[:], qf[:])
                nc.vector.tensor_copy(kb[:], kf[:])

                # Vaug[k, 0:64] = V, col 64 = 1.0 (denominator), 65:96 zeros
                # (96 rows so the O^T xbar transpose is 32-aligned).
                va = wpool.tile([128, NT, 96], bf16, tag="va")
                nc.gpsimd.memset(va[:], 0.0)
                nc.vector.tensor_copy(va[:, :, 0:D], vf[:])
                nc.gpsimd.memset(va[:, :, D : D + 1], 1.0)

                # qt[0:64, j, p] = Q^T of tile j (queries j*128+p), dims on
                # partitions; qt[64:128, j, p] = Q^T of tile j+8.
                qt = wpool.tile([128, 8, 128], bf16, tag="qt")
                kt_t = wpool.tile([128, 8, 128], bf16, tag="kt")
                for j in range(8):
                    nc.sync.dma_start(
                        qt[:, j : j + 1, :], qb[:, j : j + 1, :, :], transpose=True
                    )
                    nc.sync.dma_start(
                        kt_t[:, j : j + 1, :], kb[:, j : j + 1, :, :], transpose=True
                    )
                # [64, 1024] flat views: cols = queries 0:1024 / 1024:2048
                qt_lo = qt[0:64].rearrange("p a b -> p (a b)")
                qt_hi = qt[64:128].rearrange("p a b -> p (a b)")

                # K tiles 6,7 are needed as lhsT in the bottom partition half
                # for the query windows that cross the 1024 seam (matmul
                # requires lhsT/rhs at the same base partition). Restage as
                # pairs (7,6) and (6,7) and xbar-transpose; the transposed
                # bottom half holds the second tile of each pair:
                # kseam[64:128, 0] = tile 6 dims, kseam[64:128, 1] = tile 7.
                ks2 = wpool.tile([128, 4, D], bf16, tag="ks2")
                nc.sync.dma_start(ks2[:, 0 : 1, :], kb[:, 7 : 8, 0 : 1, :])
                nc.sync.dma_start(ks2[:, 1 : 2, :], kb[:, 6 : 7, 0 : 1, :])
                nc.sync.dma_start(ks2[:, 2 : 3, :], kb[:, 6 : 7, 0 : 1, :])
                nc.sync.dma_start(ks2[:, 3 : 4, :], kb[:, 7 : 8, 0 : 1, :])
                kseam = wpool.tile([128, 2, 128], bf16, tag="kseam")
                for j in range(2):
                    nc.sync.dma_start(
                        kseam[:, j : j + 1, :],
                        ks2[:, 2 * j : 2 * j + 2, :],
                        transpose=True,
                    )

                def qk_segments(kt, nkt):
                    """(lhsT, rhs, ncols) segments for key-tile kt covering
                    global queries [kt*128, kt*128 + nkt)."""
                    lo, hi = kt * 128, kt * 128 + nkt
                    segs = []
                    if lo < 1024:
                        c = min(hi, 1024)
                        lhsT = kt_t[0:64, kt : kt + 1, :]
                        segs.append((lhsT, qt_lo[:, lo:c], c - lo))
                    if hi > 1024:
                        c = max(lo, 1024)
                        if kt >= 8:
                            lhsT = kt_t[64:128, kt - 8 : kt - 7, :]
                        else:  # kt in (6, 7): seam copies, bottom half
                            lhsT = kseam[64:128, kt - 6 : kt - 5, :]
                        segs.append((lhsT, qt_hi[:, c - 1024 : hi - 1024], hi - c))
                    return segs

                # ---- scores + exp per key-tile ----
                pb = ppool.tile([128, NT, 384], bf16, tag="p")
                for kt in range(NT):
                    nkt = min(384, N - kt * 128)
                    s = spool.tile([128, 512], f32, tag="s")
                    col = 0
                    for lhsT, rhs, ncols in qk_segments(kt, nkt):
                        nc.tensor.matmul(
                            s[:, col : col + ncols],
                            lhsT=lhsT,
                            rhs=rhs,
                            start=(col == 0),
                            stop=(col + ncols == nkt),
                        )
                        col += ncols
                    nc.scalar.activation(
                        pb[:, kt : kt + 1, 0:nkt], s[:, 0:nkt], Exp, scale=SCALE
                    )

                # ---- band masks (batched across tiles) ----
                nc.vector.tensor_mul(
                    pb[:, :, 0:128],
                    pb[:, :, 0:128],
                    m1[:].unsqueeze(1).broadcast_to([128, NT, 128]),
                )
                # tiles 14,15 have <=256 valid cols; their [256:384) region is
                # never-written garbage and never read by PV, masking it is a no-op
                # numerically but would propagate NaN*0; restrict to tiles 0..13.
                nc.vector.tensor_mul(
                    pb[:, 0 : NT - 2, 256:384],
                    pb[:, 0 : NT - 2, 256:384],
                    m2[:].unsqueeze(1).broadcast_to([128, NT - 2, 128]),
                )

                # ---- PV with overlapped-window accumulation ----
                ops = opsum_pool.tile([128, N // 512, 512], f32, tag="o")
                for w in range(N // 512):
                    w0 = 512 * w
                    kts = [
                        kt
                        for kt in range(max(0, 4 * w - 2), min(NT, 4 * w + 4))
                        if max(kt * 128, w0) < min(kt * 128 + min(384, N - kt * 128), w0 + 512)
                    ]
                    for i, kt in enumerate(kts):
                        nkt = min(384, N - kt * 128)
                        lo = max(kt * 128, w0)
                        hi = min(kt * 128 + nkt, w0 + 512)
                        nc.tensor.matmul(
                            ops[0:96, w : w + 1, lo - w0 : hi - w0],
                            lhsT=va[:, kt : kt + 1, :],
                            rhs=pb[:, kt : kt + 1, lo - kt * 128 : hi - kt * 128],
                            start=(i == 0),
                            stop=(i == len(kts) - 1),
                        )

                # ---- evacuate, transpose back, normalize ----
                ot = opool.tile([96, N], bf16, tag="ot")
                nc.vector.tensor_copy(ot[:], ops[0:96, :, :])
                oq = opool.tile([128, NT, 96], bf16, tag="oq")
                for t in range(NT):
                    nc.sync.dma_start(
                        oq[:, t : t + 1, :],
                        ot[:, t * 128 : (t + 1) * 128],
                        transpose=True,
                    )
                r = opool.tile([128, NT], f32, tag="r")
                nc.vector.reciprocal(r[:], oq[:, :, D : D + 1].rearrange("p a b -> p (a b)"))
                out = opool.tile([128, NT, D], f32, tag="out")
                nc.vector.tensor_mul(
                    out[:],
                    oq[:, :, 0:D],
                    r[:].unsqueeze(2).broadcast_to([128, NT, D]),
                )
                nc.sync.dma_start(ov[b : b + 1], out[:])

    nc.compile()
    return nc


def kernel(q, k, v, mask=None):
    from concourse import bass_utils

    q = np.asarray(q, dtype=np.float32).reshape(B * H, N, D)
    k = np.asarray(k, dtype=np.float32).reshape(B * H, N, D)
    v = np.asarray(v, dtype=np.float32).reshape(B * H, N, D)

    if "nc" not in _CACHE:
        _CACHE["nc"] = _build()
    nc = _CACHE["nc"]

    in_maps = []
    for c in range(NCORES):
        sl = slice(c * PER, (c + 1) * PER)
        in_maps.append(
            {
                "q": np.ascontiguousarray(q[sl]),
                "k": np.ascontiguousarray(k[sl]),
                "v": np.ascontiguousarray(v[sl]),
            }
        )
    res = bass_utils.run_bass_kernel_spmd(nc, in_maps, core_ids=list(range(NCORES)))
    out = np.concatenate([res.results[c]["o"] for c in range(NCORES)], axis=0)
    return out.reshape(B, H, N, D)
